# revision 1
# baseline (speedup 1.0000x reference)
"""DecoderRNN (LSTM + Bahdanau attention + vocab projection) on 8 Trainium2 cores.

Sharding: tensor-parallel over feature dims. Core j owns:
  - hidden-unit slice Hj = [128j, 128j+128) -> z-columns for all 4 LSTM gates
  - attention-dim slice Aj = [128j, 128j+128)
  - vocab slice Vj = [4000j, 4000j+4000)
Per step: AllGather(h-shard, bf16) + AllGather(attention-score partials).
The small x_low = gate * awe path is computed fully on every core (diag-matmul
trick for awe). The vocab GEMM (bf16, SBUF-resident Wo shard) fills PE gaps.
h is carried in bf16 (ring of 4 step-slots); c stays f32 per-core.
"""
import os
import numpy as np
import ml_dtypes

B, T, E, H, V, L = 64, 24, 512, 1024, 32000, 25
A = H
NC = 8
HS = H // NC          # 128 hidden shard
VS = V // NC          # 4000 vocab shard
NPAIR = T // 2        # 12 vocab row-pairs
QL = 13               # ceil(L/2) l-pairs for awe

_BF = ml_dtypes.bfloat16


def _build(debug=False):
    import concourse.bass as bass
    import concourse.mybir as mybir
    import concourse.tile as tile
    from concourse import bacc
    from concourse.masks import make_identity

    fp32 = mybir.dt.float32
    bf16 = mybir.dt.bfloat16
    AF = mybir.ActivationFunctionType
    ALU = mybir.AluOpType

    nc = bacc.Bacc("TRN2", target_bir_lowering=False)
    din = {}

    def dram_in(name, shape, dt=fp32):
        din[name] = nc.dram_tensor(name, shape, dt, kind="ExternalInput")
        return din[name]

    # ---- inputs (per-core; host prepares layouts) ----
    t_embT = dram_in("embT", [128, 4, L * B], bf16)       # [p,a_hi,(b,l)] e=a*128+p
    t_embRM2 = dram_in("embRM2", [128, QL, E], bf16)      # [p,q,e] row=(b=p%64,l=2q+p//64)
    t_We = dram_in("We_j", [128, 4, 128], bf16)           # lhsT tiles of We[:,Aj]
    t_abias = dram_in("abias_j", [128, 1])                # be[Aj]+bd[Aj]
    t_Wd = dram_in("Wd_j", [128, 8, 128], bf16)           # lhsT tiles of Wd[:,Aj]
    t_va = dram_in("va_j", [128, 1], bf16)
    t_Wfb = dram_in("Wfb", [128, 8, E], bf16)             # full Wfb rhs tiles
    t_bfb = dram_in("bfb_row", [1, E])
    t_Wh = dram_in("Wh_j", [128, 8, 512], bf16)           # lhsT tiles of Wh[:,zcols_j]
    t_Wilo = dram_in("Wilo_j", [128, 4, 512], bf16)       # lhsT tiles of Wi[512:,zcols_j]
    t_Wihi = dram_in("Wihi_j", [128, 4, 512], bf16)       # lhsT tiles of Wi[:512,zcols_j]
    t_bl = dram_in("bl_j", [128, 4])                      # bl[zcols_j] col-chunked
    t_Wih = dram_in("Wih", [128, 4, H], bf16)             # full Wih lhsT tiles
    t_bih = dram_in("bih_c", [128, 8])
    t_Wic = dram_in("Wic_j", [128, 4, 128])               # f32 lhsT tiles of Wic[:,Hj]
    t_bic = dram_in("bic_j", [128, 1])
    t_Wo = dram_in("Wo_j", [128, 8, VS], bf16)            # rhs tiles: [p,kt,n]
    t_bo = dram_in("bo_j", [1, VS], bf16)
    t_eye = dram_in("eye2", [128, 64])                    # [I64;I64] f32

    t_out = nc.dram_tensor("logits_j", [NPAIR * 128, VS], fp32, kind="ExternalOutput")
    dbg = {}
    if debug:
        for nm, shp, dt in [("d_h0", [128, 8 * 64], bf16), ("d_att1", [128, L * B], bf16),
                        ("d_S0", [64, 25], fp32), ("d_alpha0", [128, 25], fp32),
                        ("d_xlw0", [64, 512], fp32), ("d_h1", [128, 8 * 64], bf16),
                        ("d_c1", [128, 64], fp32), ("d_zf0", [128, 4 * 64], fp32),
                        ("d_mean", [128, 4 * 64], fp32),
                        ("d_gate0", [64, 512], fp32)]:
            dbg[nm] = nc.dram_tensor(nm, shp, dt, kind="ExternalOutput")

    rg = [list(range(NC))]

    with tile.TileContext(nc) as tc:
        with (
            tc.tile_pool(name="persist", bufs=1) as pp,
            tc.tile_pool(name="work", bufs=2) as wk,
            tc.tile_pool(name="psum", bufs=1, space="PSUM") as ps,
            tc.tile_pool(name="psv", bufs=2, space="PSUM") as psv,
            tc.tile_pool(name="dram", bufs=3, space="DRAM") as dr,
        ):
            # ---------------- persistent tiles ----------------
            embRM2 = pp.tile([128, QL, E], bf16)
            att1T = pp.tile([128, B, L], bf16)
            Wd = pp.tile([128, 8, 128], bf16)
            va = pp.tile([128, 1], bf16)
            Wfb = pp.tile([128, 8, E], bf16)
            bfb = pp.tile([1, E], fp32)
            Wh = pp.tile([128, 8, 512], bf16)
            Wilo = pp.tile([128, 4, 512], bf16)
            blc = pp.tile([128, 4], fp32)
            Wo = pp.tile([128, 8, VS], bf16)
            boRep = pp.tile([128, VS], bf16)
            eye2 = pp.tile([128, 64], fp32)
            ones64 = pp.tile([1, 64], fp32)
            id64 = pp.tile([64, 64], fp32)
            cT = pp.tile([128, 64], fp32)          # own c shard
            Hbf = pp.tile([128, 8, 4, 64], bf16)   # h ring: slot s%4 holds h(s)
            alpha2 = pp.tile([128, 25], fp32)
            zpreT = pp.tile([128, 4, B, T], bf16)

            for tl, src in [(embRM2, t_embRM2), (Wd, t_Wd), (va, t_va),
                            (Wfb, t_Wfb), (bfb, t_bfb), (Wh, t_Wh),
                            (Wilo, t_Wilo), (Wo, t_Wo), (eye2, t_eye),
                            (blc, t_bl)]:
                nc.sync.dma_start(tl[:], src[:])
            nc.vector.memset(ones64[:], 1.0)
            make_identity(nc, id64[:])
            nc.vector.memset(alpha2[:], 0.0)  # col 24 of upper half must stay 0

            # ---------------- init phase ----------------
            with tc.tile_pool(name="init", bufs=1) as ip:
                embT = ip.tile([128, 4, L * B], bf16)
                We = ip.tile([128, 4, 128], bf16)
                Wih = ip.tile([128, 4, H], bf16)
                Wic = ip.tile([128, 4, 128], fp32)
                Wihi = ip.tile([128, 4, 512], bf16)
                abias = ip.tile([128, 1], fp32)
                bihc = ip.tile([128, 8], fp32)
                bic = ip.tile([128, 1], fp32)
                for tl, src in [(embT, t_embT), (We, t_We), (Wih, t_Wih),
                                (Wic, t_Wic), (Wihi, t_Wihi), (abias, t_abias),
                                (bihc, t_bih), (bic, t_bic)]:
                    nc.sync.dma_start(tl[:], src[:])
                bo_b = ip.tile([1, VS], bf16)
                nc.sync.dma_start(bo_b[:], t_bo[:])
                nc.gpsimd.partition_broadcast(boRep[:], bo_b[:])

                # att1T = (embeds @ We_j + be_j + bd_j)^T : [A_j=128, (b,l)]
                att1f = att1T.rearrange("p b l -> p (b l)")
                for c0 in range(0, L * B, 512):
                    n = min(512, L * B - c0)
                    pa = ps.tile([128, 512], fp32, tag="pA")
                    for kt in range(4):
                        nc.tensor.matmul(pa[:, 0:n], We[:, kt, :],
                                         embT[:, kt, c0:c0 + n],
                                         start=(kt == 0), stop=(kt == 3))
                    nc.scalar.activation(att1f[:, c0:c0 + n], pa[:, 0:n],
                                         AF.Identity, bias=abias[:])

                # mean_e^T [128,(4,64)]
                meanT = ip.tile([128, 4, 64], fp32)
                nc.vector.tensor_reduce(
                    meanT[:], embT.rearrange("p a (b l) -> p a b l", l=L),
                    axis=mybir.AxisListType.X, op=ALU.add)
                nc.vector.tensor_scalar_mul(meanT[:], meanT[:], 1.0 / L)
                meanB = ip.tile([128, 4, 64], bf16)
                nc.vector.tensor_copy(meanB[:], meanT[:])

                # h0 (full, bf16 into ring slot 0) / c0 (own shard, f32)
                for ch in range(8):
                    ph = ps.tile([128, 64], fp32, tag="pA")
                    for kt in range(4):
                        nc.tensor.matmul(ph[:], Wih[:, kt, ch * 128:(ch + 1) * 128],
                                         meanB[:, kt, :],
                                         start=(kt == 0), stop=(kt == 3))
                    nc.scalar.activation(Hbf[:, ch, 0, :], ph[:], AF.Identity,
                                         bias=bihc[:, ch:ch + 1])
                pc = ps.tile([128, 64], fp32, tag="pA")
                for kt in range(4):
                    nc.tensor.matmul(pc[:], Wic[:, kt, :], meanT[:, kt, :],
                                     start=(kt == 0), stop=(kt == 3))
                nc.scalar.activation(cT[:], pc[:], AF.Identity, bias=bic[:])

                # zpre[t] = emb_t @ Wi_hi[:, zcols_j] + bl_j  (all t, chunked)
                rhs4 = embT.rearrange("p a (b l) -> p a b l", l=L)[:, :, :, 0:T]
                for ch in range(4):
                    for b0 in range(0, B, 16):
                        pzc = ps.tile([128, 16, T], fp32, tag="pA")
                        for kt in range(4):
                            nc.tensor.matmul(pzc[:],
                                             Wihi[:, kt, ch * 128:(ch + 1) * 128],
                                             rhs4[:, kt, b0:b0 + 16, :],
                                             start=(kt == 0), stop=(kt == 3))
                        nc.scalar.activation(zpreT[:, ch, b0:b0 + 16, :], pzc[:],
                                             AF.Identity, bias=blc[:, ch:ch + 1])

                if debug:
                    nc.sync.dma_start(dbg["d_h0"].rearrange("p (k b) -> p k b", b=64),
                                      Hbf[:, :, 0, :])
                    nc.sync.dma_start(dbg["d_att1"][:], att1f[:])
                    nc.sync.dma_start(dbg["d_mean"][:],
                                      meanT.rearrange("p a b -> p (a b)"))

            # ---------------- step loop ----------------
            for t in range(T):
                s_cur = t % 4
                s_nxt = (t + 1) % 4
                # hd = h @ Wd_j -> psum [128(a), 64(b)]
                phd = ps.tile([128, 64], fp32, tag="pA")
                for kt in range(8):
                    nc.tensor.matmul(phd[:], Wd[:, kt, :], Hbf[:, kt, s_cur, :],
                                     start=(kt == 0), stop=(kt == 7))
                # R = relu(att1T + hd)  [128, b, l] bf16
                R = wk.tile([128, B, L], bf16, tag="R")
                nc.vector.tensor_tensor(R[:], att1T[:],
                                        phd[:, :, None].broadcast_to([128, 64, L]),
                                        ALU.add)
                nc.vector.tensor_scalar_max(R[:], R[:], 0.0)
                # score partial = va_j^T R -> psum [4, 512] (flat 1600 as q*512+n)
                Rf = R.rearrange("p b l -> p (b l)")
                psc = ps.tile([128, 512], fp32, tag="psc")
                for q in range(4):
                    n = min(512, L * B - q * 512)
                    nc.tensor.matmul(psc[32 * q:32 * q + 1, 0:n], va[:],
                                     Rf[:, q * 512:q * 512 + n],
                                     start=True, stop=True,
                                     tile_position=(0, 32 * q))
                scS = wk.tile([97, 512], fp32, tag="scS")
                nc.vector.tensor_copy(scS[:], psc[0:97, :])
                cc_sin = dr.tile([4, 512], fp32, tag="cc_sin")
                cc_sout = dr.tile([NC, 4 * 512], fp32, tag="cc_sout")
                nc.sync.dma_start(cc_sin[:], scS[0:97:32, :])
                nc.gpsimd.collective_compute(
                    "AllGather", ALU.bypass, replica_groups=rg,
                    ins=[cc_sin.opt()], outs=[cc_sout.opt()])
                # S[b,l] = sum_j partials
                Sg = wk.tile([64, NC, L], fp32, tag="Sg")
                nc.sync.dma_start(
                    Sg[:],
                    cc_sout[:, 0:L * B].rearrange("j (b l) -> b j l", l=L))
                S = wk.tile([64, 25], fp32, tag="S")
                nc.vector.tensor_reduce(S[:], Sg.rearrange("b j l -> b l j"),
                                        axis=mybir.AxisListType.X, op=ALU.add)
                # softmax (no max-sub; scores are small)
                Zt = wk.tile([64, 1], fp32, tag="Zt")
                nc.scalar.activation(alpha2[0:64, :], S[:], AF.Exp, accum_out=Zt[:])
                nc.scalar.activation(alpha2[64:128, 0:24], S[:, 1:25], AF.Exp)
                Zr = wk.tile([128, 1], fp32, tag="Zr")
                nc.vector.reciprocal(Zr[0:64, :], Zt[:])
                nc.vector.tensor_copy(Zr[64:128, :], Zr[0:64, :])
                alphaN = wk.tile([128, 25], fp32, tag="alphaN")
                nc.vector.tensor_scalar(alphaN[:], alpha2[:], Zr[:], None, ALU.mult)
                # DmatAll2 [128, q, 64] bf16 = eye2 * alphaN[:, 2q (+1 upper)]
                Dm = wk.tile([128, QL, 64], bf16, tag="Dm")
                a_v = alphaN[:, 0:25:2][:, :, None].broadcast_to([128, QL, 64])
                e_v = eye2[:, None, :].broadcast_to([128, QL, 64])
                nc.vector.tensor_tensor(Dm[:], e_v, a_v, ALU.mult)
                # awe (row-major, full): psum [64, 512]
                pawe = ps.tile([64, E], fp32, tag="pawe")
                for q in range(QL):
                    nc.tensor.matmul(pawe[:], Dm[:, q, :], embRM2[:, q, :],
                                     start=(q == 0), stop=(q == QL - 1))
                # gate (row-major, full): psum [64, 512]
                pgate = ps.tile([64, E], fp32, tag="pgate")
                for kt in range(8):
                    nc.tensor.matmul(pgate[:], Hbf[:, kt, s_cur, :], Wfb[:, kt, :],
                                     start=(kt == 0), stop=False)
                nc.tensor.matmul(pgate[:], ones64[:], bfb[:], start=False, stop=True)
                gateS = wk.tile([64, E], fp32, tag="gateS")
                nc.scalar.activation(gateS[:], pgate[:], AF.Sigmoid)
                xlw = wk.tile([64, E], fp32, tag="xlw")
                nc.vector.tensor_tensor(xlw[:], gateS[:], pawe[:], ALU.mult)
                # transpose x_low -> xT bf16 [128, 4, 64]
                pxT = ps.tile([128, 4, 64], fp32, tag="pA")
                for q in range(4):
                    nc.tensor.transpose(pxT[:, q, :], xlw[:, q * 128:(q + 1) * 128],
                                        id64[:])
                xT = wk.tile([128, 4, 64], bf16, tag="xT")
                nc.scalar.copy(xT[:], pxT[:])
                # z = x @ Wi + h @ Wh (own z-cols) : psum [128, 4, 64]
                pz = ps.tile([128, 4, 64], fp32, tag="pz")
                for ch in range(4):
                    for kt in range(8):
                        nc.tensor.matmul(pz[:, ch, :],
                                         Wh[:, kt, ch * 128:(ch + 1) * 128],
                                         Hbf[:, kt, s_cur, :],
                                         start=(kt == 0), stop=False)
                    for kt in range(4):
                        nc.tensor.matmul(pz[:, ch, :],
                                         Wilo[:, kt, ch * 128:(ch + 1) * 128],
                                         xT[:, kt, :],
                                         start=False, stop=(kt == 3))
                zf = wk.tile([128, 4, 64], fp32, tag="zf")
                nc.vector.tensor_tensor(zf[:], pz[:], zpreT[:, :, :, t], ALU.add)
                # gates: order i,f,g,o along ch
                gsb = wk.tile([128, 4, 64], fp32, tag="gsb")
                nc.scalar.activation(gsb[:, 0:2, :], zf[:, 0:2, :], AF.Sigmoid)
                nc.scalar.activation(gsb[:, 2, :], zf[:, 2, :], AF.Tanh)
                nc.scalar.activation(gsb[:, 3, :], zf[:, 3, :], AF.Sigmoid)
                ig = wk.tile([128, 64], fp32, tag="ig")
                nc.vector.tensor_tensor(ig[:], gsb[:, 0, :], gsb[:, 2, :], ALU.mult)
                fc = wk.tile([128, 64], fp32, tag="fc")
                nc.vector.tensor_tensor(fc[:], gsb[:, 1, :], cT[:], ALU.mult)
                nc.vector.tensor_tensor(cT[:], fc[:], ig[:], ALU.add)
                tc_t = wk.tile([128, 64], fp32, tag="tc_t")
                nc.scalar.activation(tc_t[:], cT[:], AF.Tanh)
                hO = wk.tile([128, 64], bf16, tag="hO")
                nc.vector.tensor_tensor(hO[:], gsb[:, 3, :], tc_t[:], ALU.mult)
                # AllGather h (bf16)
                cc_hin = dr.tile([128, 64], bf16, tag="cc_hin")
                cc_hout = dr.tile([H, 64], bf16, tag="cc_hout")
                nc.sync.dma_start(cc_hin[:], hO[:])
                nc.gpsimd.collective_compute(
                    "AllGather", ALU.bypass, replica_groups=rg,
                    ins=[cc_hin.opt()], outs=[cc_hout.opt()])
                nc.sync.dma_start(Hbf[:, :, s_nxt, :],
                                  cc_hout.rearrange("(k p) b -> p k b", p=128))

                if debug and t == 0:
                    nc.sync.dma_start(dbg["d_S0"][:], S[:])
                    nc.sync.dma_start(dbg["d_alpha0"][:], alphaN[:])
                    nc.sync.dma_start(dbg["d_xlw0"][:], xlw[:])
                    nc.sync.dma_start(dbg["d_gate0"][:], gateS[:])
                    nc.sync.dma_start(dbg["d_zf0"][:],
                                      zf.rearrange("p c b -> p (c b)"))
                    nc.sync.dma_start(dbg["d_c1"][:], cT[:])
                    nc.sync.dma_start(dbg["d_h1"].rearrange("p (k b) -> p k b", b=64),
                                      Hbf[:, :, s_nxt, :])

                # vocab GEMM for pair (h(t), h(t+1)) at odd t -> logits rows
                if t % 2 == 1:
                    m = t // 2
                    Hv = wk.tile([128, 8, 2, 64], bf16, tag="Hv")
                    nc.vector.tensor_copy(Hv[:, :, 0, :], Hbf[:, :, s_cur, :])
                    nc.vector.tensor_copy(Hv[:, :, 1, :], Hbf[:, :, s_nxt, :])
                    for ns in range(8):
                        n0 = ns * 500
                        pv = psv.tile([128, 500], fp32, tag="pv")
                        for kt in range(8):
                            lhs = Hv[:, kt, :, :].rearrange("p s b -> p (s b)")
                            nc.tensor.matmul(pv[:], lhs, Wo[:, kt, n0:n0 + 500],
                                             start=(kt == 0), stop=(kt == 7))
                        lg = wk.tile([128, 500], fp32, tag="lg")
                        nc.vector.tensor_tensor(lg[:], pv[:],
                                                boRep[:, n0:n0 + 500], ALU.add)
                        nc.sync.dma_start(
                            t_out[m * 128:(m + 1) * 128, n0:n0 + 500], lg[:])

    nc.finalize()
    return nc


def _host_prep(features, captions, emb, We, be, Wd, bd, va, ba,
               Wih, bih, Wic, bic, Wfb, bfb, Wi, Wh, bl, Wo, bo):
    """Build the 8 per-core input maps (layout/sharding only)."""
    f32 = np.float32
    embeds = np.concatenate([features[:, None, :], emb[captions]], 1)  # [B,L,E]
    flatE = np.ascontiguousarray(embeds.reshape(B * L, E), dtype=f32)
    embT = np.ascontiguousarray(
        flatE.T.reshape(4, 128, B * L).transpose(1, 0, 2)).astype(_BF)
    p = np.arange(128)
    qi = np.arange(QL)
    l_idx = 2 * qi[None, :] + (p // 64)[:, None]
    b_idx = (p % 64)[:, None].repeat(QL, 1)
    valid = l_idx < L
    embRM2 = np.zeros((128, QL, E), f32)
    embRM2[valid] = embeds[b_idx[valid], l_idx[valid]]
    embRM2 = embRM2.astype(_BF)

    def lhsT_tiles(w):  # [K, M] -> [128, K//128, M]
        K, M = w.shape
        return np.ascontiguousarray(w.reshape(K // 128, 128, M).transpose(1, 0, 2))

    eye2 = np.vstack([np.eye(64, dtype=f32)] * 2)
    in_maps = []
    for j in range(NC):
        hs = slice(128 * j, 128 * j + 128)
        zcols = np.concatenate([np.arange(128) + 1024 * g + 128 * j
                                for g in range(4)])
        vsl = slice(VS * j, VS * (j + 1))
        m = {
            "embT": embT.reshape(128, 4, B * L),
            "embRM2": embRM2,
            "We_j": lhsT_tiles(We[:, hs]).astype(_BF),
            "abias_j": (be[hs] + bd[hs]).reshape(128, 1).astype(f32),
            "Wd_j": lhsT_tiles(Wd[:, hs]).astype(_BF),
            "va_j": va[hs].reshape(128, 1).astype(_BF),
            "Wfb": lhsT_tiles(Wfb).astype(_BF),
            "bfb_row": bfb.reshape(1, E).astype(f32),
            "Wh_j": lhsT_tiles(Wh[:, zcols]).astype(_BF),
            "Wilo_j": lhsT_tiles(Wi[512:, zcols]).astype(_BF),
            "Wihi_j": lhsT_tiles(Wi[:512, zcols]).astype(_BF),
            "bl_j": np.ascontiguousarray(bl[zcols].reshape(4, 128).T).astype(f32),
            "Wih": lhsT_tiles(Wih).astype(_BF),
            "bih_c": np.ascontiguousarray(bih.reshape(8, 128).T).astype(f32),
            "Wic_j": lhsT_tiles(Wic[:, hs]).astype(f32),
            "bic_j": bic[hs].reshape(128, 1).astype(f32),
            "Wo_j": lhsT_tiles(Wo[:, vsl]).astype(_BF),
            "bo_j": bo[vsl].reshape(1, VS).astype(_BF),
            "eye2": eye2,
        }
        in_maps.append(m)
    return in_maps


_CACHE = {}


def kernel(**inputs):
    from concourse.bass_utils import run_bass_kernel_spmd

    inputs = {k: np.asarray(v) for k, v in inputs.items()}
    in_maps = _host_prep(
        inputs["features"], inputs["captions"], inputs["emb"], inputs["We"],
        inputs["be"], inputs["Wd"], inputs["bd"], inputs["va"], inputs["ba"],
        inputs["Wih"], inputs["bih"], inputs["Wic"], inputs["bic"],
        inputs["Wfb"], inputs["bfb"], inputs["Wi"], inputs["Wh"], inputs["bl"],
        inputs["Wo"], inputs["bo"])
    debug = bool(int(os.environ.get("KDBG", "0")))
    if "nc" not in _CACHE or _CACHE.get("debug") != debug:
        _CACHE["nc"] = _build(debug=debug)
        _CACHE["debug"] = debug
    res = run_bass_kernel_spmd(_CACHE["nc"], in_maps, core_ids=list(range(NC)))
    if debug:
        _CACHE["dbg"] = res.results
    outs = [r["logits_j"] for r in res.results]
    full = np.concatenate(
        [o.reshape(NPAIR, 2, 64, VS) for o in outs], axis=3)  # [12,2,64,V]
    return np.ascontiguousarray(
        full.transpose(2, 0, 1, 3).reshape(64, T, V)).astype(np.float32)

